# revision 49
# baseline (speedup 1.0000x reference)
"""Trainium2 Bass kernel for a 4-layer RNN stack (LSTM x2 + GRU x2, H=64) + FC head.

Strategy: data-parallel over batch (B=2048 -> 256/core on 8 cores). On each
core, all four layers run as a software wavefront over time: at global step s,
layer L processes timestep t = s - L. All state lives on-chip.

Per step: 14 gate matmuls (self/below K-split; L0's x and all biases ride
extra rhs rows — x is DMA'd into Hh rows 64-65 as [ones; x] each step),
per-chunk sigmoids plus a real tanh for the LSTM g-gate, one combined
[f*c; i*u] fp16 DVE multiply against a persistent [c; u] SBUF tile, and
identity/selector matmuls for the cross-partition folds (LSTM c' = f*c + i*u,
GRU n_pre = q - m*hnb). Engine queues are ordered so the h_L recurrence (the
critical loop) threads first; GRU self-MMs fill the PE while the LSTM tail
runs. Gate pre-activations accumulate in fp32 PSUM; element-wise state fp16.
"""

import numpy as np

H = 64
B_CORE = 256
T_FULL = 512
N_CORES = 8
NBLK = 19  # weight blocks of 128 cols each

f16 = np.float16


# ---------------------------------------------------------------------------
# Host-side weight packing
# ---------------------------------------------------------------------------
def _pack_weights(inp):
    """Build the [128, NBLK*128] fp16 lhsT table. Block j lives at cols 128j.

    LSTM chunk A = [f; i] rows, chunk B = [o; 2*g] rows (sigma(2g) trick).
    GRU  chunk A = [z; -r] rows, chunk B = [q; hnb] (q = xn-part + hn-part).
    """
    W = np.zeros((128, NBLK * 128), np.float32)

    def put(j, arr):
        a = np.asarray(arr, np.float32)
        W[: a.shape[0], 128 * j : 128 * j + a.shape[1]] = a

    def lstm_blocks(w_ih, w_hh, b_ih, b_hh, j0, first):
        b = np.asarray(b_ih, np.float32) + np.asarray(b_hh, np.float32)
        w_ih = np.asarray(w_ih, np.float32)
        w_hh = np.asarray(w_hh, np.float32)
        rA = list(range(64, 128)) + list(range(0, 64))      # [f; i]
        rB = list(range(192, 256)) + list(range(128, 192))  # [o; g]
        for k, rows in enumerate([rA, rB]):
            whh = w_hh[rows]
            wih = w_ih[rows]
            bb = b[rows]
            if first:
                # L0: one merged MM; x rides Hh row 65 (ones stays at 64)
                xr = wih[:, 0][None, :]
                put(j0 + 2 * k, np.vstack([whh.T, bb[None, :], xr]))  # [66,128]
            else:
                put(j0 + 2 * k, whh.T)                                # [64,128]
                put(j0 + 2 * k + 1, np.vstack([wih.T, bb[None, :]]))  # [65,128]

    def gru_parts(w_ih, w_hh, b_ih, b_hh, flip):
        """Return (selfA, belowA+bias, selfB, belowB+bias) for one GRU.

        Normal (G0): A cols = [z; -r], B cols = [q; hnb], row-half LOW.
        Flipped (G1): A cols = [-r; z], B cols = [hnb; q], row-half HIGH.
        """
        w_ih = np.asarray(w_ih, np.float32)
        w_hh = np.asarray(w_hh, np.float32)
        b_ih = np.asarray(b_ih, np.float32)
        b_hh = np.asarray(b_hh, np.float32)
        r, z, n = slice(0, 64), slice(64, 128), slice(128, 192)
        zc = (w_hh[z], w_ih[z], b_ih[z] + b_hh[z])
        rc = (-w_hh[r], -w_ih[r], -(b_ih[r] + b_hh[r]))
        qc = (w_hh[n], w_ih[n], b_ih[n] + b_hh[n])
        hc = (w_hh[n], np.zeros_like(w_ih[n]), b_hh[n])
        a0, a1 = (rc, zc) if flip else (zc, rc)
        b0, b1 = (hc, qc) if flip else (qc, hc)
        selfA = np.concatenate([a0[0], a1[0]], axis=0).T
        belowA = np.vstack([np.concatenate([a0[1], a1[1]], axis=0).T,
                            np.concatenate([a0[2], a1[2]])[None, :]])
        selfB = np.concatenate([b0[0], b1[0]], axis=0).T
        belowB = np.vstack([np.concatenate([b0[1], b1[1]], axis=0).T,
                            np.concatenate([b0[2], b1[2]])[None, :]])
        return selfA, belowA, selfB, belowB

    lstm_blocks(inp["lw_ih0"], inp["lw_hh0"], inp["lb_ih0"], inp["lb_hh0"], 0, True)
    lstm_blocks(inp["lw_ih1"], inp["lw_hh1"], inp["lb_ih1"], inp["lb_hh1"], 4, False)
    sA0, bA0, sB0, bB0 = gru_parts(
        inp["gw_ih0"], inp["gw_hh0"], inp["gb_ih0"], inp["gb_hh0"], False)
    sA1, bA1, sB1, bB1 = gru_parts(
        inp["gw_ih1"], inp["gw_hh1"], inp["gb_ih1"], inp["gb_hh1"], True)
    # merged self blocks: rows 0-63 contract g0h (LOW), 64-127 g1h (HIGH)
    put(8, np.vstack([sA0, sA1]))             # [128,128]
    put(9, bA0)                               # [65,128] G0A below+bias
    put(10, np.vstack([sB0, sB1]))            # [128,128]
    put(11, bB0)                              # [65,128] G0B below+bias
    put(13, bA1)                              # [65,128] G1A below+bias
    put(15, bB1)                              # [65,128] G1B below+bias

    eye = np.eye(64, dtype=np.float32)
    put(16, np.vstack([eye, eye]))            # IDT2: out[m] = x[m] + x[m+64]
    z64 = np.zeros((64, 64), np.float32)
    # NSEL2: out[m<64] = -x[m+64], out[m>=64] = -x[m-64]
    put(17, np.block([[z64, -eye], [-eye, z64]]))
    fc = np.zeros((128, 128), np.float32)
    fc[64:128, 0] = np.asarray(inp["fc_w"], np.float32)[0]  # g1h on HIGH rows
    put(18, fc)
    fcb = np.zeros((65, 128), np.float32)
    fcb[64, 0] = np.asarray(inp["fc_b"], np.float32)[0]     # rides ones row
    put(12, fcb)
    return W.astype(f16)


# ---------------------------------------------------------------------------
# Bass kernel
# ---------------------------------------------------------------------------
def build_kernel(T):
    import concourse.bass as bass
    import concourse.tile as tile
    from concourse import mybir, bacc

    dt = mybir.dt
    Alu = mybir.AluOpType
    Act = mybir.ActivationFunctionType
    STEPS = T + 3

    nc = bacc.Bacc(None, target_bir_lowering=False, debug=False)
    xt = nc.dram_tensor("xt", [2 * STEPS, 256], dt.float16, kind="ExternalInput").ap()
    wts = nc.dram_tensor("wts", [128, NBLK * 128], dt.float16, kind="ExternalInput").ap()
    y = nc.dram_tensor("y", [1, 256], dt.float32, kind="ExternalOutput").ap()

    def blk(j, k=128, w=128):
        return (slice(0, k), slice(128 * j, 128 * j + w))

    with tile.TileContext(nc) as tc:
        with (
            tc.tile_pool(name="state", bufs=1) as state,
            tc.tile_pool(name="work", bufs=3) as work,
            tc.tile_pool(name="psigL", bufs=1, space="PSUM") as psigL,
            tc.tile_pool(name="psigG", bufs=2, space="PSUM") as psigG,
            tc.tile_pool(name="pgq", bufs=2, space="PSUM") as pgq,
            tc.tile_pool(name="pcs", bufs=1, space="PSUM") as pcs,
        ):
            WT = state.tile([128, NBLK * 128], dt.float16)
            Hh = state.tile([66, 1024], dt.float16)
            HG = state.tile([128, 512], dt.float16)  # [g0h|0 ; 0|g1h]
            PtG = state.tile([128, 512], dt.float16)
            Ut = state.tile([128, 512], dt.float16)  # rows 0-63: c; 64-127: u
            gq = pgq.tile([128, 512], dt.float32)   # G0:[q;hnb] | G1:[hnb;q]
            pc = pcs.tile([128, 512], dt.float32)   # c state L0|L1 (rows 0-63)

            nc.sync.dma_start(WT[:], wts)
            nc.vector.memset(Hh[0:64, :], 0.0)
            nc.vector.memset(Hh[64:66, :], 1.0)     # ones + x slot
            nc.vector.memset(HG[:, :], 0.0)         # zero-blocks load-bearing
            nc.vector.memset(pc[0:64, :], 0.0)      # c state init
            nc.vector.memset(Ut[0:64, :], 0.0)      # c fp16 mirror init

            for s in range(STEPS):
                sigL = psigL.tile([128, 1024], dt.float32, tag="sigL")
                sigG = psigG.tile([128, 512], dt.float32, tag="sigG")
                SHL = work.tile([128, 1024], dt.float16, tag="shl")
                SHG = work.tile([128, 512], dt.float16, tag="shg")
                PtL = work.tile([128, 512], dt.float16, tag="ptl")
                THc = work.tile([64, 512], dt.float16, tag="thc")
                THn = work.tile([128, 512], dt.float16, tag="thn")
                Dt = work.tile([128, 512], dt.float16, tag="dt")
                Et = work.tile([128, 512], dt.float16, tag="et")

                # x for step s rides Hh row 65 ([ones; x] DMA'd to rows 64-65)
                nc.sync.dma_start(Hh[64:66, 0:256], xt[2 * s : 2 * s + 2, :])

                mm = nc.tensor.matmul
                # --- LSTM gate matmuls first: ready at h_L, the critical loop ---
                mm(sigL[:, 0:256], WT[blk(0, 66)], Hh[0:66, 0:256],
                   start=True, stop=True, skip_group_check=True)
                mm(sigL[:, 256:512], WT[blk(4, 64)], Hh[0:64, 256:512],
                   start=False, stop=False, skip_group_check=True)
                mm(sigL[:, 256:512], WT[blk(5, 65)], Hh[0:65, 0:256],
                   start=False, stop=True, skip_group_check=True)
                mm(sigL[:, 512:768], WT[blk(2, 66)], Hh[0:66, 0:256],
                   start=True, stop=True, skip_group_check=True)
                mm(sigL[:, 768:1024], WT[blk(6, 64)], Hh[0:64, 256:512],
                   start=False, stop=False, skip_group_check=True)
                mm(sigL[:, 768:1024], WT[blk(7, 65)], Hh[0:65, 0:256],
                   start=False, stop=True, skip_group_check=True)

                # --- merged GRU self matmuls (block-diag over [g0h; g1h]) ---
                mm(sigG[:, 0:512], WT[blk(8, 128)], HG[:, :],
                   start=True, stop=False, skip_group_check=True)
                mm(gq[:, 0:512], WT[blk(10, 128)], HG[:, :],
                   start=True, stop=False, skip_group_check=True)

                # --- GRU chunk-A below matmuls ---
                mm(sigG[:, 0:256], WT[blk(9, 65)], Hh[0:65, 256:512],
                   start=False, stop=True, skip_group_check=True)
                mm(sigG[:, 256:512], WT[blk(13, 65)], Hh[0:65, 512:768],
                   start=False, stop=True, skip_group_check=True)

                # --- activations in readiness order ---
                nc.scalar.activation(SHL[:, 0:512], sigL[:, 0:512], Act.Sigmoid)
                nc.scalar.activation(Ut[64:128, :], sigL[64:128, 512:1024],
                                     Act.Tanh)
                nc.scalar.activation(SHL[0:64, 512:1024], sigL[0:64, 512:1024],
                                     Act.Sigmoid)
                nc.scalar.activation(SHG[:], sigG[:, :], Act.Sigmoid)

                # --- LSTM element-wise: [f*c; i*u] in one 2x fp16 op ---
                nc.vector.tensor_tensor(PtL[:, :], SHL[:, 0:512],
                                        Ut[:, :], Alu.mult)

                # c' = f*c + i*u (cross-partition fold on PE)
                mm(pc[0:64, :], WT[blk(16, 128, 64)], PtL[:, :],
                   start=True, stop=True, skip_group_check=True)
                nc.scalar.activation(THc[:, :], pc[0:64, :], Act.Tanh)

                # --- GRU chunk-B below matmuls (feed PtG/NSEL later) ---
                mm(gq[:, 0:256], WT[blk(11, 65)], Hh[0:65, 256:512],
                   start=False, stop=False, skip_group_check=True)
                mm(gq[:, 256:512], WT[blk(15, 65)], Hh[0:65, 512:768],
                   start=False, stop=False, skip_group_check=True)

                # --- GRU element-wise: t = m*hnb; n_pre = q - t ---
                # meaningful: G0 rows 64-127, G1 rows 0-63 (flipped layout)
                nc.vector.tensor_tensor(PtG[:, :], SHG[:, :],
                                        gq[:, 0:512], Alu.mult)
                mm(gq[:, 0:512], WT[blk(17, 128, 128)], PtG[:, :],
                   start=False, stop=True, skip_group_check=True)
                nc.scalar.activation(THn[:, :], gq[:, 0:512], Act.Tanh)

                # h_lstm = o * tanh(c)  (feeds the critical loop first)
                nc.vector.tensor_tensor(Hh[0:64, 0:512], SHL[0:64, 512:1024],
                                        THc[:, :], Alu.mult)
                # GRU tail: d = h_prev - n ; e = z*d ; h' = n + e
                # (off-quadrant lanes compute finite garbage; never read)
                nc.vector.tensor_tensor(Dt[:, :], HG[:, :],
                                        THn[:, :], Alu.subtract)
                nc.vector.tensor_tensor(Et[:, :], SHG[:, :],
                                        Dt[:, :], Alu.mult)
                nc.vector.tensor_tensor(HG[0:64, 0:256], THn[0:64, 0:256],
                                        Et[0:64, 0:256], Alu.add)
                nc.vector.tensor_tensor(HG[64:128, 256:512],
                                        THn[64:128, 256:512],
                                        Et[64:128, 256:512], Alu.add)
                # evacuate c' to the fp16 [c; u] tile (needed only next step;
                # on ScalarE so it stays out of the contended DVE queue)
                nc.scalar.activation(Ut[0:64, :], pc[0:64, :], Act.Copy)
                # G1 reads g0h (+ones) from Hh; keep that copy in sync
                nc.gpsimd.tensor_copy(Hh[0:64, 512:768], HG[0:64, 0:256])

                # wavefront warm-up: re-zero states of layers not yet active
                if s == 0:
                    nc.vector.memset(Hh[0:64, 256:512], 0.0)      # h1
                    nc.vector.memset(pc[0:64, 256:512], 0.0)      # c1
                    nc.vector.memset(Ut[0:64, 256:512], 0.0)      # c1 mirror
                elif s == 1:
                    nc.vector.memset(Hh[0:64, 512:768], 0.0)      # g0h
                    nc.vector.memset(HG[0:64, 0:256], 0.0)
                elif s == 2:
                    nc.vector.memset(HG[64:128, 256:512], 0.0)    # g1h

            # --- FC head: y = g1h(T-1) @ fc_w.T + fc_b ---
            ysb = state.tile([1, 256], dt.float32)
            nc.tensor.matmul(pc[0:1, 0:256], WT[blk(18, 128, 1)],
                             HG[:, 256:512],
                             start=True, stop=False, skip_group_check=True)
            nc.tensor.matmul(pc[0:1, 0:256],
                             WT[64:65, 128 * 12 : 128 * 12 + 1],
                             Hh[64:65, 0:256],
                             start=False, stop=True, skip_group_check=True)
            nc.vector.tensor_copy(ysb[:], pc[0:1, 0:256])
            nc.sync.dma_start(y, ysb[:])

    nc.compile()
    return nc


_NC_CACHE = {}


def _get_nc(T):
    if T not in _NC_CACHE:
        _NC_CACHE[T] = build_kernel(T)
    return _NC_CACHE[T]


def _prep_inputs(inputs, T):
    from concourse import bass_utils  # noqa: F401

    x = np.asarray(inputs["x"], np.float32)  # [2048, T_FULL, 1]
    W = _pack_weights(inputs)
    STEPS = T + 3
    in_maps = []
    for c in range(N_CORES):
        xc = x[c * B_CORE : (c + 1) * B_CORE, :T, 0]      # [256, T]
        xtc = np.zeros((STEPS, 2, 256), f16)
        xtc[:, 0, :] = 1.0                                # ones row refresh
        xtc[:T, 1, :] = xc.T.astype(f16)
        in_maps.append({"xt": xtc.reshape(2 * STEPS, 256), "wts": W})
    return in_maps


def run(inputs, T):
    from concourse import bass_utils

    nc = _get_nc(T)
    in_maps = _prep_inputs(inputs, T)
    res = bass_utils.run_bass_kernel_spmd(nc, in_maps, core_ids=list(range(N_CORES)))
    out = np.concatenate([res.results[c]["y"].reshape(256, 1) for c in range(N_CORES)], axis=0)
    return out.astype(np.float32)


def kernel(**inputs):
    return run(inputs, T_FULL)



# revision 50
# speedup vs baseline: 1.1999x; 1.1999x over previous
"""Trainium2 Bass kernel for a 4-layer RNN stack (LSTM x2 + GRU x2, H=64) + FC head.

Strategy: data-parallel over batch (B=2048 -> 256/core on 8 cores). On each
core, all four layers run as a software wavefront over time: at global step s,
layer L processes timestep t = s - L. All state lives on-chip.

Per step: 14 gate matmuls (self/below K-split; L0's x and all biases ride
extra rhs rows — x is DMA'd into Hh rows 64-65 as [ones; x] each step),
per-chunk sigmoids plus a real tanh for the LSTM g-gate, one combined
[f*c; i*u] fp16 DVE multiply against a persistent [c; u] SBUF tile, and
identity/selector matmuls for the cross-partition folds (LSTM c' = f*c + i*u,
GRU n_pre = q - m*hnb). Engine queues are ordered so the h_L recurrence (the
critical loop) threads first; GRU self-MMs fill the PE while the LSTM tail
runs. Gate pre-activations accumulate in fp32 PSUM; element-wise state fp16.
"""

import numpy as np

H = 64
B_CORE = 256
T_FULL = 512
N_CORES = 8
NBLK = 19  # weight blocks of 128 cols each

f16 = np.float16


# ---------------------------------------------------------------------------
# Host-side weight packing
# ---------------------------------------------------------------------------
def _pack_weights(inp):
    """Build the [128, NBLK*128] fp16 lhsT table. Block j lives at cols 128j.

    LSTM chunk A = [f; i] rows, chunk B = [o; 2*g] rows (sigma(2g) trick).
    GRU  chunk A = [z; -r] rows, chunk B = [q; hnb] (q = xn-part + hn-part).
    """
    W = np.zeros((128, NBLK * 128), np.float32)

    def put(j, arr):
        a = np.asarray(arr, np.float32)
        W[: a.shape[0], 128 * j : 128 * j + a.shape[1]] = a

    def lstm_blocks(w_ih, w_hh, b_ih, b_hh, j0, first):
        b = np.asarray(b_ih, np.float32) + np.asarray(b_hh, np.float32)
        w_ih = np.asarray(w_ih, np.float32)
        w_hh = np.asarray(w_hh, np.float32)
        rA = list(range(64, 128)) + list(range(0, 64))      # [f; i]
        rB = list(range(192, 256)) + list(range(128, 192))  # [o; g]
        for k, rows in enumerate([rA, rB]):
            whh = w_hh[rows]
            wih = w_ih[rows]
            bb = b[rows]
            if first:
                # L0: one merged MM; x rides Hh row 65 (ones stays at 64)
                xr = wih[:, 0][None, :]
                put(j0 + 2 * k, np.vstack([whh.T, bb[None, :], xr]))  # [66,128]
            else:
                put(j0 + 2 * k, whh.T)                                # [64,128]
                put(j0 + 2 * k + 1, np.vstack([wih.T, bb[None, :]]))  # [65,128]

    def gru_parts(w_ih, w_hh, b_ih, b_hh, flip):
        """Return (selfA, belowA+bias, selfB, belowB+bias) for one GRU.

        Normal (G0): A cols = [z; -r], B cols = [q; hnb], row-half LOW.
        Flipped (G1): A cols = [-r; z], B cols = [hnb; q], row-half HIGH.
        """
        w_ih = np.asarray(w_ih, np.float32)
        w_hh = np.asarray(w_hh, np.float32)
        b_ih = np.asarray(b_ih, np.float32)
        b_hh = np.asarray(b_hh, np.float32)
        r, z, n = slice(0, 64), slice(64, 128), slice(128, 192)
        zc = (w_hh[z], w_ih[z], b_ih[z] + b_hh[z])
        rc = (-w_hh[r], -w_ih[r], -(b_ih[r] + b_hh[r]))
        qc = (w_hh[n], w_ih[n], b_ih[n] + b_hh[n])
        hc = (w_hh[n], np.zeros_like(w_ih[n]), b_hh[n])
        a0, a1 = (rc, zc) if flip else (zc, rc)
        b0, b1 = (hc, qc) if flip else (qc, hc)
        selfA = np.concatenate([a0[0], a1[0]], axis=0).T
        belowA = np.vstack([np.concatenate([a0[1], a1[1]], axis=0).T,
                            np.concatenate([a0[2], a1[2]])[None, :]])
        selfB = np.concatenate([b0[0], b1[0]], axis=0).T
        belowB = np.vstack([np.concatenate([b0[1], b1[1]], axis=0).T,
                            np.concatenate([b0[2], b1[2]])[None, :]])
        return selfA, belowA, selfB, belowB

    lstm_blocks(inp["lw_ih0"], inp["lw_hh0"], inp["lb_ih0"], inp["lb_hh0"], 0, True)
    lstm_blocks(inp["lw_ih1"], inp["lw_hh1"], inp["lb_ih1"], inp["lb_hh1"], 4, False)
    sA0, bA0, sB0, bB0 = gru_parts(
        inp["gw_ih0"], inp["gw_hh0"], inp["gb_ih0"], inp["gb_hh0"], False)
    sA1, bA1, sB1, bB1 = gru_parts(
        inp["gw_ih1"], inp["gw_hh1"], inp["gb_ih1"], inp["gb_hh1"], True)
    # merged self blocks: rows 0-63 contract g0h (LOW), 64-127 g1h (HIGH)
    put(8, np.vstack([sA0, sA1]))             # [128,128]
    put(9, bA0)                               # [65,128] G0A below+bias
    put(10, np.vstack([sB0, sB1]))            # [128,128]
    put(11, bB0)                              # [65,128] G0B below+bias
    put(13, bA1)                              # [65,128] G1A below+bias
    put(15, bB1)                              # [65,128] G1B below+bias

    eye = np.eye(64, dtype=np.float32)
    put(16, np.vstack([eye, eye]))            # IDT2: out[m] = x[m] + x[m+64]
    z64 = np.zeros((64, 64), np.float32)
    # NSEL2: out[m<64] = -x[m+64], out[m>=64] = -x[m-64]
    put(17, np.block([[z64, -eye], [-eye, z64]]))
    fc = np.zeros((128, 128), np.float32)
    fc[64:128, 0] = np.asarray(inp["fc_w"], np.float32)[0]  # g1h on HIGH rows
    put(18, fc)
    fcb = np.zeros((65, 128), np.float32)
    fcb[64, 0] = np.asarray(inp["fc_b"], np.float32)[0]     # rides ones row
    put(12, fcb)
    return W.astype(f16)


# ---------------------------------------------------------------------------
# Bass kernel
# ---------------------------------------------------------------------------
def build_kernel(T):
    import concourse.bass as bass
    import concourse.tile as tile
    from concourse import mybir, bacc

    dt = mybir.dt
    Alu = mybir.AluOpType
    Act = mybir.ActivationFunctionType
    STEPS = T + 3

    nc = bacc.Bacc(None, target_bir_lowering=False, debug=False)
    xt = nc.dram_tensor("xt", [2 * STEPS, 256], dt.float16, kind="ExternalInput").ap()
    wts = nc.dram_tensor("wts", [128, NBLK * 128], dt.float16, kind="ExternalInput").ap()
    y = nc.dram_tensor("y", [1, 256], dt.float32, kind="ExternalOutput").ap()

    def blk(j, k=128, w=128):
        return (slice(0, k), slice(128 * j, 128 * j + w))

    with tile.TileContext(nc) as tc:
        with (
            tc.tile_pool(name="state", bufs=1) as state,
            tc.tile_pool(name="work", bufs=3) as work,
            tc.tile_pool(name="psigL", bufs=1, space="PSUM") as psigL,
            tc.tile_pool(name="psigG", bufs=2, space="PSUM") as psigG,
            tc.tile_pool(name="pgq", bufs=2, space="PSUM") as pgq,
            tc.tile_pool(name="pcs", bufs=1, space="PSUM") as pcs,
        ):
            WT = state.tile([128, NBLK * 128], dt.float16)
            Hh = state.tile([66, 1024], dt.float16)
            HG = state.tile([128, 512], dt.float16)  # [g0h|0 ; 0|g1h]
            PtG = state.tile([128, 512], dt.float16)
            Ut = state.tile([128, 512], dt.float16)  # rows 0-63: c; 64-127: u
            gq = pgq.tile([128, 512], dt.float32)   # G0:[q;hnb] | G1:[hnb;q]
            pc = pcs.tile([128, 512], dt.float32)   # c state L0|L1 (rows 0-63)

            nc.sync.dma_start(WT[:], wts)
            nc.vector.memset(Hh[0:64, :], 0.0)
            nc.vector.memset(Hh[64:66, :], 1.0)     # ones + x slot
            nc.vector.memset(HG[:, :], 0.0)         # zero-blocks load-bearing
            nc.vector.memset(pc[0:64, :], 0.0)      # c state init
            nc.vector.memset(Ut[0:64, :], 0.0)      # c fp16 mirror init

            for s in range(STEPS):
                sigL = psigL.tile([128, 1024], dt.float32, tag="sigL")
                sigG = psigG.tile([128, 512], dt.float32, tag="sigG")
                SHL = work.tile([128, 1024], dt.float16, tag="shl")
                SHG = work.tile([128, 512], dt.float16, tag="shg")
                PtL = work.tile([128, 512], dt.float16, tag="ptl")
                THc = work.tile([64, 512], dt.float16, tag="thc")
                THn = work.tile([128, 512], dt.float16, tag="thn")
                Dt = work.tile([128, 512], dt.float16, tag="dt")
                Et = work.tile([128, 512], dt.float16, tag="et")

                # x for step s rides Hh row 65 ([ones; x] DMA'd to rows 64-65)
                nc.sync.dma_start(Hh[64:66, 0:256], xt[2 * s : 2 * s + 2, :])

                mm = nc.tensor.matmul
                # --- LSTM gate matmuls first: ready at h_L, the critical loop ---
                mm(sigL[:, 0:256], WT[blk(0, 66)], Hh[0:66, 0:256],
                   start=True, stop=True, skip_group_check=True)
                mm(sigL[:, 256:512], WT[blk(4, 64)], Hh[0:64, 256:512],
                   start=False, stop=False, skip_group_check=True)
                mm(sigL[:, 256:512], WT[blk(5, 65)], Hh[0:65, 0:256],
                   start=False, stop=True, skip_group_check=True)
                mm(sigL[:, 512:768], WT[blk(2, 66)], Hh[0:66, 0:256],
                   start=True, stop=True, skip_group_check=True)
                mm(sigL[:, 768:1024], WT[blk(6, 64)], Hh[0:64, 256:512],
                   start=False, stop=False, skip_group_check=True)
                mm(sigL[:, 768:1024], WT[blk(7, 65)], Hh[0:65, 0:256],
                   start=False, stop=True, skip_group_check=True)

                # --- merged GRU self matmuls (block-diag over [g0h; g1h]) ---
                mm(sigG[:, 0:512], WT[blk(8, 128)], HG[:, :],
                   start=True, stop=False, skip_group_check=True)
                mm(gq[:, 0:512], WT[blk(10, 128)], HG[:, :],
                   start=True, stop=False, skip_group_check=True)

                # --- GRU chunk-A below matmuls ---
                mm(sigG[:, 0:256], WT[blk(9, 65)], Hh[0:65, 256:512],
                   start=False, stop=True, skip_group_check=True)
                mm(sigG[:, 256:512], WT[blk(13, 65)], Hh[0:65, 512:768],
                   start=False, stop=True, skip_group_check=True)

                # --- activations in readiness order ---
                nc.scalar.activation(SHL[:, 0:512], sigL[:, 0:512], Act.Sigmoid)
                nc.scalar.activation(Ut[64:128, :], sigL[64:128, 512:1024],
                                     Act.Tanh)
                nc.scalar.activation(SHL[0:64, 512:1024], sigL[0:64, 512:1024],
                                     Act.Sigmoid)
                nc.scalar.activation(SHG[:], sigG[:, :], Act.Sigmoid)

                # --- LSTM element-wise: [f*c; i*u] in one 2x fp16 op ---
                nc.vector.tensor_tensor(PtL[:, :], SHL[:, 0:512],
                                        Ut[:, :], Alu.mult)

                # c' = f*c + i*u (cross-partition fold on PE)
                mm(pc[0:64, :], WT[blk(16, 128, 64)], PtL[:, :],
                   start=True, stop=True, skip_group_check=True)
                nc.scalar.activation(THc[:, :], pc[0:64, :], Act.Tanh)

                # --- GRU chunk-B below matmuls (feed PtG/NSEL later) ---
                mm(gq[:, 0:256], WT[blk(11, 65)], Hh[0:65, 256:512],
                   start=False, stop=False, skip_group_check=True)
                mm(gq[:, 256:512], WT[blk(15, 65)], Hh[0:65, 512:768],
                   start=False, stop=False, skip_group_check=True)

                # --- GRU element-wise: t = m*hnb; n_pre = q - t ---
                # meaningful: G0 rows 64-127, G1 rows 0-63 (flipped layout)
                nc.vector.tensor_tensor(PtG[:, :], SHG[:, :],
                                        gq[:, 0:512], Alu.mult)
                mm(gq[:, 0:512], WT[blk(17, 128, 128)], PtG[:, :],
                   start=False, stop=True, skip_group_check=True)
                nc.scalar.activation(THn[:, :], gq[:, 0:512], Act.Tanh)

                # h_lstm = o * tanh(c)  (feeds the critical loop first)
                nc.vector.tensor_tensor(Hh[0:64, 0:512], SHL[0:64, 512:1024],
                                        THc[:, :], Alu.mult)
                # GRU tail: d = h_prev - n ; e = z*d ; h' = n + e
                # (off-quadrant lanes compute finite garbage; never read)
                nc.vector.tensor_tensor(Dt[:, :], HG[:, :],
                                        THn[:, :], Alu.subtract)
                nc.vector.tensor_tensor(Et[:, :], SHG[:, :],
                                        Dt[:, :], Alu.mult)
                nc.vector.tensor_tensor(HG[0:64, 0:256], THn[0:64, 0:256],
                                        Et[0:64, 0:256], Alu.add)
                nc.vector.tensor_tensor(HG[64:128, 256:512],
                                        THn[64:128, 256:512],
                                        Et[64:128, 256:512], Alu.add)
                # evacuate c' to the fp16 [c; u] tile (needed only next step,
                # so it must not sit ahead of h_lstm in the DVE queue)
                nc.vector.tensor_copy(Ut[0:64, :], pc[0:64, :])
                # G1 reads g0h (+ones) from Hh; keep that copy in sync
                nc.gpsimd.tensor_copy(Hh[0:64, 512:768], HG[0:64, 0:256])

                # wavefront warm-up: re-zero states of layers not yet active
                if s == 0:
                    nc.vector.memset(Hh[0:64, 256:512], 0.0)      # h1
                    nc.vector.memset(pc[0:64, 256:512], 0.0)      # c1
                    nc.vector.memset(Ut[0:64, 256:512], 0.0)      # c1 mirror
                elif s == 1:
                    nc.vector.memset(Hh[0:64, 512:768], 0.0)      # g0h
                    nc.vector.memset(HG[0:64, 0:256], 0.0)
                elif s == 2:
                    nc.vector.memset(HG[64:128, 256:512], 0.0)    # g1h

            # --- FC head: y = g1h(T-1) @ fc_w.T + fc_b ---
            ysb = state.tile([1, 256], dt.float32)
            nc.tensor.matmul(pc[0:1, 0:256], WT[blk(18, 128, 1)],
                             HG[:, 256:512],
                             start=True, stop=False, skip_group_check=True)
            nc.tensor.matmul(pc[0:1, 0:256],
                             WT[64:65, 128 * 12 : 128 * 12 + 1],
                             Hh[64:65, 0:256],
                             start=False, stop=True, skip_group_check=True)
            nc.vector.tensor_copy(ysb[:], pc[0:1, 0:256])
            nc.sync.dma_start(y, ysb[:])

    nc.compile()
    return nc


_NC_CACHE = {}


def _get_nc(T):
    if T not in _NC_CACHE:
        _NC_CACHE[T] = build_kernel(T)
    return _NC_CACHE[T]


def _prep_inputs(inputs, T):
    from concourse import bass_utils  # noqa: F401

    x = np.asarray(inputs["x"], np.float32)  # [2048, T_FULL, 1]
    W = _pack_weights(inputs)
    STEPS = T + 3
    in_maps = []
    for c in range(N_CORES):
        xc = x[c * B_CORE : (c + 1) * B_CORE, :T, 0]      # [256, T]
        xtc = np.zeros((STEPS, 2, 256), f16)
        xtc[:, 0, :] = 1.0                                # ones row refresh
        xtc[:T, 1, :] = xc.T.astype(f16)
        in_maps.append({"xt": xtc.reshape(2 * STEPS, 256), "wts": W})
    return in_maps


def run(inputs, T):
    from concourse import bass_utils

    nc = _get_nc(T)
    in_maps = _prep_inputs(inputs, T)
    res = bass_utils.run_bass_kernel_spmd(nc, in_maps, core_ids=list(range(N_CORES)))
    out = np.concatenate([res.results[c]["y"].reshape(256, 1) for c in range(N_CORES)], axis=0)
    return out.astype(np.float32)


def kernel(**inputs):
    return run(inputs, T_FULL)



# revision 51
# speedup vs baseline: 1.2184x; 1.0154x over previous
"""Trainium2 Bass kernel for a 4-layer RNN stack (LSTM x2 + GRU x2, H=64) + FC head.

Strategy: data-parallel over batch (B=2048 -> 256/core on 8 cores). On each
core, all four layers run as a software wavefront over time: at global step s,
layer L processes timestep t = s - L. All state lives on-chip.

Per step: 14 gate matmuls (self/below K-split; L0's x and all biases ride
extra rhs rows — x is DMA'd into Hh rows 64-65 as [ones; x] each step),
per-chunk sigmoids plus a real tanh for the LSTM g-gate, one combined
[f*c; i*u] fp16 DVE multiply against a persistent [c; u] SBUF tile, and
identity/selector matmuls for the cross-partition folds (LSTM c' = f*c + i*u,
GRU n_pre = q - m*hnb). Engine queues are ordered so the h_L recurrence (the
critical loop) threads first; GRU self-MMs fill the PE while the LSTM tail
runs. Gate pre-activations accumulate in fp32 PSUM; element-wise state fp16.
"""

import numpy as np

H = 64
B_CORE = 256
T_FULL = 512
N_CORES = 8
NBLK = 19  # weight blocks of 128 cols each

f16 = np.float16


# ---------------------------------------------------------------------------
# Host-side weight packing
# ---------------------------------------------------------------------------
def _pack_weights(inp):
    """Build the [128, NBLK*128] fp16 lhsT table. Block j lives at cols 128j.

    LSTM chunk A = [f; i] rows, chunk B = [o; 2*g] rows (sigma(2g) trick).
    GRU  chunk A = [z; -r] rows, chunk B = [q; hnb] (q = xn-part + hn-part).
    """
    W = np.zeros((128, NBLK * 128), np.float32)

    def put(j, arr):
        a = np.asarray(arr, np.float32)
        W[: a.shape[0], 128 * j : 128 * j + a.shape[1]] = a

    def lstm_blocks(w_ih, w_hh, b_ih, b_hh, j0, first):
        b = np.asarray(b_ih, np.float32) + np.asarray(b_hh, np.float32)
        w_ih = np.asarray(w_ih, np.float32)
        w_hh = np.asarray(w_hh, np.float32)
        rA = list(range(64, 128)) + list(range(0, 64))      # [f; i]
        rB = list(range(192, 256)) + list(range(128, 192))  # [o; g]
        for k, rows in enumerate([rA, rB]):
            whh = w_hh[rows]
            wih = w_ih[rows]
            bb = b[rows]
            if first:
                # L0: one merged MM; x rides Hh row 65 (ones stays at 64)
                xr = wih[:, 0][None, :]
                put(j0 + 2 * k, np.vstack([whh.T, bb[None, :], xr]))  # [66,128]
            else:
                put(j0 + 2 * k, whh.T)                                # [64,128]
                put(j0 + 2 * k + 1, np.vstack([wih.T, bb[None, :]]))  # [65,128]

    def gru_parts(w_ih, w_hh, b_ih, b_hh, flip):
        """Return (selfA, belowA+bias, selfB, belowB+bias) for one GRU.

        Normal (G0): A cols = [z; -r], B cols = [q; hnb], row-half LOW.
        Flipped (G1): A cols = [-r; z], B cols = [hnb; q], row-half HIGH.
        """
        w_ih = np.asarray(w_ih, np.float32)
        w_hh = np.asarray(w_hh, np.float32)
        b_ih = np.asarray(b_ih, np.float32)
        b_hh = np.asarray(b_hh, np.float32)
        r, z, n = slice(0, 64), slice(64, 128), slice(128, 192)
        zc = (w_hh[z], w_ih[z], b_ih[z] + b_hh[z])
        rc = (-w_hh[r], -w_ih[r], -(b_ih[r] + b_hh[r]))
        qc = (w_hh[n], w_ih[n], b_ih[n] + b_hh[n])
        hc = (w_hh[n], np.zeros_like(w_ih[n]), b_hh[n])
        a0, a1 = (rc, zc) if flip else (zc, rc)
        b0, b1 = (hc, qc) if flip else (qc, hc)
        selfA = np.concatenate([a0[0], a1[0]], axis=0).T
        belowA = np.vstack([np.concatenate([a0[1], a1[1]], axis=0).T,
                            np.concatenate([a0[2], a1[2]])[None, :]])
        selfB = np.concatenate([b0[0], b1[0]], axis=0).T
        belowB = np.vstack([np.concatenate([b0[1], b1[1]], axis=0).T,
                            np.concatenate([b0[2], b1[2]])[None, :]])
        return selfA, belowA, selfB, belowB

    lstm_blocks(inp["lw_ih0"], inp["lw_hh0"], inp["lb_ih0"], inp["lb_hh0"], 0, True)
    lstm_blocks(inp["lw_ih1"], inp["lw_hh1"], inp["lb_ih1"], inp["lb_hh1"], 4, False)
    sA0, bA0, sB0, bB0 = gru_parts(
        inp["gw_ih0"], inp["gw_hh0"], inp["gb_ih0"], inp["gb_hh0"], False)
    sA1, bA1, sB1, bB1 = gru_parts(
        inp["gw_ih1"], inp["gw_hh1"], inp["gb_ih1"], inp["gb_hh1"], True)
    # merged self blocks: rows 0-63 contract g0h (LOW), 64-127 g1h (HIGH)
    put(8, np.vstack([sA0, sA1]))             # [128,128]
    put(9, bA0)                               # [65,128] G0A below+bias
    put(10, np.vstack([sB0, sB1]))            # [128,128]
    put(11, bB0)                              # [65,128] G0B below+bias
    put(13, bA1)                              # [65,128] G1A below+bias
    put(15, bB1)                              # [65,128] G1B below+bias

    eye = np.eye(64, dtype=np.float32)
    put(16, np.vstack([eye, eye]))            # IDT2: out[m] = x[m] + x[m+64]
    z64 = np.zeros((64, 64), np.float32)
    # NSEL2: out[m<64] = -x[m+64], out[m>=64] = -x[m-64]
    put(17, np.block([[z64, -eye], [-eye, z64]]))
    fc = np.zeros((128, 128), np.float32)
    fc[64:128, 0] = np.asarray(inp["fc_w"], np.float32)[0]  # g1h on HIGH rows
    put(18, fc)
    fcb = np.zeros((65, 128), np.float32)
    fcb[64, 0] = np.asarray(inp["fc_b"], np.float32)[0]     # rides ones row
    put(12, fcb)
    return W.astype(f16)


# ---------------------------------------------------------------------------
# Bass kernel
# ---------------------------------------------------------------------------
def build_kernel(T):
    import concourse.bass as bass
    import concourse.tile as tile
    from concourse import mybir, bacc

    dt = mybir.dt
    Alu = mybir.AluOpType
    Act = mybir.ActivationFunctionType
    STEPS = T + 3

    nc = bacc.Bacc(None, target_bir_lowering=False, debug=False)
    xt = nc.dram_tensor("xt", [2 * STEPS, 256], dt.float16, kind="ExternalInput").ap()
    wts = nc.dram_tensor("wts", [128, NBLK * 128], dt.float16, kind="ExternalInput").ap()
    y = nc.dram_tensor("y", [1, 256], dt.float32, kind="ExternalOutput").ap()

    def blk(j, k=128, w=128):
        return (slice(0, k), slice(128 * j, 128 * j + w))

    with tile.TileContext(nc) as tc:
        with (
            tc.tile_pool(name="state", bufs=1) as state,
            tc.tile_pool(name="work", bufs=2) as work,
            tc.tile_pool(name="psigL", bufs=1, space="PSUM") as psigL,
            tc.tile_pool(name="psigG", bufs=2, space="PSUM") as psigG,
            tc.tile_pool(name="pgq", bufs=2, space="PSUM") as pgq,
            tc.tile_pool(name="pcs", bufs=1, space="PSUM") as pcs,
        ):
            WT = state.tile([128, NBLK * 128], dt.float16)
            Hh = state.tile([66, 1024], dt.float16)
            HG = state.tile([128, 512], dt.float16)  # [g0h|0 ; 0|g1h]
            PtG = state.tile([128, 512], dt.float16)
            Ut = state.tile([128, 512], dt.float16)  # rows 0-63: c; 64-127: u
            gq = pgq.tile([128, 512], dt.float32)   # G0:[q;hnb] | G1:[hnb;q]
            pc = pcs.tile([128, 512], dt.float32)   # c state L0|L1 (rows 0-63)

            nc.sync.dma_start(WT[:], wts)
            nc.vector.memset(Hh[0:64, :], 0.0)
            nc.vector.memset(Hh[64:66, :], 1.0)     # ones + x slot
            nc.vector.memset(HG[:, :], 0.0)         # zero-blocks load-bearing
            nc.vector.memset(pc[0:64, :], 0.0)      # c state init
            nc.vector.memset(Ut[0:64, :], 0.0)      # c fp16 mirror init

            for s in range(STEPS):
                sigL = psigL.tile([128, 1024], dt.float32, tag="sigL")
                sigG = psigG.tile([128, 512], dt.float32, tag="sigG")
                SHL = work.tile([128, 1024], dt.float16, tag="shl")
                SHG = work.tile([128, 512], dt.float16, tag="shg")
                PtL = work.tile([128, 512], dt.float16, tag="ptl")
                THc = work.tile([64, 512], dt.float16, tag="thc")
                THn = work.tile([128, 512], dt.float16, tag="thn")
                Dt = work.tile([128, 512], dt.float16, tag="dt")
                Et = work.tile([128, 512], dt.float16, tag="et")

                # x for step s rides Hh row 65 ([ones; x] DMA'd to rows 64-65)
                nc.sync.dma_start(Hh[64:66, 0:256], xt[2 * s : 2 * s + 2, :])

                mm = nc.tensor.matmul
                # --- merged GRU self matmuls (block-diag over [g0h; g1h]) ---
                mm(sigG[:, 0:512], WT[blk(8, 128)], HG[:, :],
                   start=True, stop=False, skip_group_check=True)
                mm(gq[:, 0:512], WT[blk(10, 128)], HG[:, :],
                   start=True, stop=False, skip_group_check=True)

                # --- LSTM gate matmuls: the h_L recurrence is critical ---
                mm(sigL[:, 0:256], WT[blk(0, 66)], Hh[0:66, 0:256],
                   start=True, stop=True, skip_group_check=True)
                mm(sigL[:, 256:512], WT[blk(4, 64)], Hh[0:64, 256:512],
                   start=False, stop=False, skip_group_check=True)
                mm(sigL[:, 256:512], WT[blk(5, 65)], Hh[0:65, 0:256],
                   start=False, stop=True, skip_group_check=True)
                mm(sigL[:, 512:768], WT[blk(2, 66)], Hh[0:66, 0:256],
                   start=True, stop=True, skip_group_check=True)
                mm(sigL[:, 768:1024], WT[blk(6, 64)], Hh[0:64, 256:512],
                   start=False, stop=False, skip_group_check=True)
                mm(sigL[:, 768:1024], WT[blk(7, 65)], Hh[0:65, 0:256],
                   start=False, stop=True, skip_group_check=True)

                # --- GRU chunk-A below matmuls ---
                mm(sigG[:, 0:256], WT[blk(9, 65)], Hh[0:65, 256:512],
                   start=False, stop=True, skip_group_check=True)
                mm(sigG[:, 256:512], WT[blk(13, 65)], Hh[0:65, 512:768],
                   start=False, stop=True, skip_group_check=True)

                # --- activations in readiness order ---
                nc.scalar.activation(SHL[:, 0:512], sigL[:, 0:512], Act.Sigmoid)
                nc.scalar.activation(Ut[64:128, :], sigL[64:128, 512:1024],
                                     Act.Tanh)
                nc.scalar.activation(SHL[0:64, 512:1024], sigL[0:64, 512:1024],
                                     Act.Sigmoid)
                nc.scalar.activation(SHG[:], sigG[:, :], Act.Sigmoid)

                # --- LSTM element-wise: [f*c; i*u] in one 2x fp16 op ---
                nc.vector.tensor_tensor(PtL[:, :], SHL[:, 0:512],
                                        Ut[:, :], Alu.mult)

                # c' = f*c + i*u (cross-partition fold on PE)
                mm(pc[0:64, :], WT[blk(16, 128, 64)], PtL[:, :],
                   start=True, stop=True, skip_group_check=True)
                nc.scalar.activation(THc[:, :], pc[0:64, :], Act.Tanh)
                # evacuate c' to the fp16 [c; u] tile for the next step
                nc.vector.tensor_copy(Ut[0:64, :], pc[0:64, :])

                # --- GRU chunk-B below matmuls (feed PtG/NSEL later) ---
                mm(gq[:, 0:256], WT[blk(11, 65)], Hh[0:65, 256:512],
                   start=False, stop=False, skip_group_check=True)
                mm(gq[:, 256:512], WT[blk(15, 65)], Hh[0:65, 512:768],
                   start=False, stop=False, skip_group_check=True)

                # --- GRU element-wise: t = m*hnb; n_pre = q - t ---
                # meaningful: G0 rows 64-127, G1 rows 0-63 (flipped layout)
                nc.vector.tensor_tensor(PtG[:, :], SHG[:, :],
                                        gq[:, 0:512], Alu.mult)
                mm(gq[:, 0:512], WT[blk(17, 128, 128)], PtG[:, :],
                   start=False, stop=True, skip_group_check=True)
                nc.scalar.activation(THn[:, :], gq[:, 0:512], Act.Tanh)

                # h_lstm = o * tanh(c)  (feeds the critical loop first)
                nc.vector.tensor_tensor(Hh[0:64, 0:512], SHL[0:64, 512:1024],
                                        THc[:, :], Alu.mult)
                # GRU tail: d = h_prev - n ; e = z*d ; h' = n + e
                # (off-quadrant lanes compute finite garbage; never read)
                nc.vector.tensor_tensor(Dt[:, :], HG[:, :],
                                        THn[:, :], Alu.subtract)
                nc.vector.tensor_tensor(Et[:, :], SHG[:, :],
                                        Dt[:, :], Alu.mult)
                nc.vector.tensor_tensor(HG[0:64, 0:256], THn[0:64, 0:256],
                                        Et[0:64, 0:256], Alu.add)
                nc.vector.tensor_tensor(HG[64:128, 256:512],
                                        THn[64:128, 256:512],
                                        Et[64:128, 256:512], Alu.add)
                # G1 reads g0h (+ones) from Hh; keep that copy in sync
                nc.vector.tensor_copy(Hh[0:64, 512:768], HG[0:64, 0:256])

                # wavefront warm-up: re-zero states of layers not yet active
                if s == 0:
                    nc.vector.memset(Hh[0:64, 256:512], 0.0)      # h1
                    nc.vector.memset(pc[0:64, 256:512], 0.0)      # c1
                    nc.vector.memset(Ut[0:64, 256:512], 0.0)      # c1 mirror
                elif s == 1:
                    nc.vector.memset(Hh[0:64, 512:768], 0.0)      # g0h
                    nc.vector.memset(HG[0:64, 0:256], 0.0)
                elif s == 2:
                    nc.vector.memset(HG[64:128, 256:512], 0.0)    # g1h

            # --- FC head: y = g1h(T-1) @ fc_w.T + fc_b ---
            ysb = state.tile([1, 256], dt.float32)
            nc.tensor.matmul(pc[0:1, 0:256], WT[blk(18, 128, 1)],
                             HG[:, 256:512],
                             start=True, stop=False, skip_group_check=True)
            nc.tensor.matmul(pc[0:1, 0:256],
                             WT[64:65, 128 * 12 : 128 * 12 + 1],
                             Hh[64:65, 0:256],
                             start=False, stop=True, skip_group_check=True)
            nc.vector.tensor_copy(ysb[:], pc[0:1, 0:256])
            nc.sync.dma_start(y, ysb[:])

    nc.compile()
    return nc


_NC_CACHE = {}


def _get_nc(T):
    if T not in _NC_CACHE:
        _NC_CACHE[T] = build_kernel(T)
    return _NC_CACHE[T]


def _prep_inputs(inputs, T):
    from concourse import bass_utils  # noqa: F401

    x = np.asarray(inputs["x"], np.float32)  # [2048, T_FULL, 1]
    W = _pack_weights(inputs)
    STEPS = T + 3
    in_maps = []
    for c in range(N_CORES):
        xc = x[c * B_CORE : (c + 1) * B_CORE, :T, 0]      # [256, T]
        xtc = np.zeros((STEPS, 2, 256), f16)
        xtc[:, 0, :] = 1.0                                # ones row refresh
        xtc[:T, 1, :] = xc.T.astype(f16)
        in_maps.append({"xt": xtc.reshape(2 * STEPS, 256), "wts": W})
    return in_maps


def run(inputs, T):
    from concourse import bass_utils

    nc = _get_nc(T)
    in_maps = _prep_inputs(inputs, T)
    res = bass_utils.run_bass_kernel_spmd(nc, in_maps, core_ids=list(range(N_CORES)))
    out = np.concatenate([res.results[c]["y"].reshape(256, 1) for c in range(N_CORES)], axis=0)
    return out.astype(np.float32)


def kernel(**inputs):
    return run(inputs, T_FULL)



# revision 53
# speedup vs baseline: 1.2193x; 1.0007x over previous
"""Trainium2 Bass kernel for a 4-layer RNN stack (LSTM x2 + GRU x2, H=64) + FC head.

Strategy: data-parallel over batch (B=2048 -> 256/core on 8 cores). On each
core, all four layers run as a software wavefront over time: at global step s,
layer L processes timestep t = s - L. All state lives on-chip.

Per step: 14 gate matmuls (self/below K-split; L0's x and all biases ride
extra rhs rows — x is DMA'd into Hh rows 64-65 as [ones; x] each step),
per-chunk sigmoids plus a real tanh for the LSTM g-gate, one combined
[f*c; i*u] fp16 DVE multiply against a persistent [c; u] SBUF tile, and
identity/selector matmuls for the cross-partition folds (LSTM c' = f*c + i*u,
GRU n_pre = q - m*hnb). Engine queues are ordered so the h_L recurrence (the
critical loop) threads first; GRU self-MMs fill the PE while the LSTM tail
runs. Gate pre-activations accumulate in fp32 PSUM; element-wise state fp16.
"""

import numpy as np

H = 64
B_CORE = 256
T_FULL = 512
N_CORES = 8
NBLK = 19  # weight blocks of 128 cols each

f16 = np.float16


# ---------------------------------------------------------------------------
# Host-side weight packing
# ---------------------------------------------------------------------------
def _pack_weights(inp):
    """Build the [128, NBLK*128] fp16 lhsT table. Block j lives at cols 128j.

    LSTM chunk A = [f; i] rows, chunk B = [o; 2*g] rows (sigma(2g) trick).
    GRU  chunk A = [z; -r] rows, chunk B = [q; hnb] (q = xn-part + hn-part).
    """
    W = np.zeros((128, NBLK * 128), np.float32)

    def put(j, arr):
        a = np.asarray(arr, np.float32)
        W[: a.shape[0], 128 * j : 128 * j + a.shape[1]] = a

    def lstm_blocks(w_ih, w_hh, b_ih, b_hh, j0, first):
        b = np.asarray(b_ih, np.float32) + np.asarray(b_hh, np.float32)
        w_ih = np.asarray(w_ih, np.float32)
        w_hh = np.asarray(w_hh, np.float32)
        rA = list(range(64, 128)) + list(range(0, 64))      # [f; i]
        rB = list(range(192, 256)) + list(range(128, 192))  # [o; g]
        for k, rows in enumerate([rA, rB]):
            whh = w_hh[rows]
            wih = w_ih[rows]
            bb = b[rows]
            if first:
                # L0: one merged MM; x rides Hh row 65 (ones stays at 64)
                xr = wih[:, 0][None, :]
                put(j0 + 2 * k, np.vstack([whh.T, bb[None, :], xr]))  # [66,128]
            else:
                put(j0 + 2 * k, whh.T)                                # [64,128]
                put(j0 + 2 * k + 1, np.vstack([wih.T, bb[None, :]]))  # [65,128]

    def gru_parts(w_ih, w_hh, b_ih, b_hh, flip):
        """Return (selfA, belowA+bias, selfB, belowB+bias) for one GRU.

        Normal (G0): A cols = [z; -r], B cols = [q; hnb], row-half LOW.
        Flipped (G1): A cols = [-r; z], B cols = [hnb; q], row-half HIGH.
        """
        w_ih = np.asarray(w_ih, np.float32)
        w_hh = np.asarray(w_hh, np.float32)
        b_ih = np.asarray(b_ih, np.float32)
        b_hh = np.asarray(b_hh, np.float32)
        r, z, n = slice(0, 64), slice(64, 128), slice(128, 192)
        zc = (w_hh[z], w_ih[z], b_ih[z] + b_hh[z])
        rc = (-w_hh[r], -w_ih[r], -(b_ih[r] + b_hh[r]))
        qc = (w_hh[n], w_ih[n], b_ih[n] + b_hh[n])
        hc = (w_hh[n], np.zeros_like(w_ih[n]), b_hh[n])
        a0, a1 = (rc, zc) if flip else (zc, rc)
        b0, b1 = (hc, qc) if flip else (qc, hc)
        selfA = np.concatenate([a0[0], a1[0]], axis=0).T
        belowA = np.vstack([np.concatenate([a0[1], a1[1]], axis=0).T,
                            np.concatenate([a0[2], a1[2]])[None, :]])
        selfB = np.concatenate([b0[0], b1[0]], axis=0).T
        belowB = np.vstack([np.concatenate([b0[1], b1[1]], axis=0).T,
                            np.concatenate([b0[2], b1[2]])[None, :]])
        return selfA, belowA, selfB, belowB

    lstm_blocks(inp["lw_ih0"], inp["lw_hh0"], inp["lb_ih0"], inp["lb_hh0"], 0, True)
    lstm_blocks(inp["lw_ih1"], inp["lw_hh1"], inp["lb_ih1"], inp["lb_hh1"], 4, False)
    sA0, bA0, sB0, bB0 = gru_parts(
        inp["gw_ih0"], inp["gw_hh0"], inp["gb_ih0"], inp["gb_hh0"], False)
    sA1, bA1, sB1, bB1 = gru_parts(
        inp["gw_ih1"], inp["gw_hh1"], inp["gb_ih1"], inp["gb_hh1"], True)
    # merged self blocks: rows 0-63 contract g0h (LOW), 64-127 g1h (HIGH)
    put(8, np.vstack([sA0, sA1]))             # [128,128]
    put(9, bA0)                               # [65,128] G0A below+bias
    put(10, np.vstack([sB0, sB1]))            # [128,128]
    put(11, bB0)                              # [65,128] G0B below+bias
    put(13, bA1)                              # [65,128] G1A below+bias
    put(15, bB1)                              # [65,128] G1B below+bias

    eye = np.eye(64, dtype=np.float32)
    put(16, np.vstack([eye, eye]))            # IDT2: out[m] = x[m] + x[m+64]
    z64 = np.zeros((64, 64), np.float32)
    # NSEL2: out[m<64] = -x[m+64], out[m>=64] = -x[m-64]
    put(17, np.block([[z64, -eye], [-eye, z64]]))
    fc = np.zeros((128, 128), np.float32)
    fc[64:128, 0] = np.asarray(inp["fc_w"], np.float32)[0]  # g1h on HIGH rows
    put(18, fc)
    fcb = np.zeros((65, 128), np.float32)
    fcb[64, 0] = np.asarray(inp["fc_b"], np.float32)[0]     # rides ones row
    put(12, fcb)
    return W.astype(f16)


# ---------------------------------------------------------------------------
# Bass kernel
# ---------------------------------------------------------------------------
def build_kernel(T):
    import concourse.bass as bass
    import concourse.tile as tile
    from concourse import mybir, bacc

    dt = mybir.dt
    Alu = mybir.AluOpType
    Act = mybir.ActivationFunctionType
    STEPS = T + 3

    nc = bacc.Bacc(None, target_bir_lowering=False, debug=False)
    xt = nc.dram_tensor("xt", [2 * STEPS, 256], dt.float16, kind="ExternalInput").ap()
    wts = nc.dram_tensor("wts", [128, NBLK * 128], dt.float16, kind="ExternalInput").ap()
    y = nc.dram_tensor("y", [1, 256], dt.float32, kind="ExternalOutput").ap()

    def blk(j, k=128, w=128):
        return (slice(0, k), slice(128 * j, 128 * j + w))

    with tile.TileContext(nc) as tc:
        with (
            tc.tile_pool(name="state", bufs=1) as state,
            tc.tile_pool(name="work", bufs=2) as work,
            tc.tile_pool(name="psigL", bufs=1, space="PSUM") as psigL,
            tc.tile_pool(name="psigG", bufs=2, space="PSUM") as psigG,
            tc.tile_pool(name="pgq", bufs=2, space="PSUM") as pgq,
            tc.tile_pool(name="pcs", bufs=1, space="PSUM") as pcs,
        ):
            WT = state.tile([128, NBLK * 128], dt.float16)
            Hh = state.tile([66, 1024], dt.float16)
            HG = state.tile([128, 512], dt.float16)  # [g0h|0 ; 0|g1h]
            PtG = state.tile([128, 512], dt.float16)
            Ut = state.tile([128, 512], dt.float16)  # rows 0-63: c; 64-127: u
            gq = pgq.tile([128, 512], dt.float32)   # G0:[q;hnb] | G1:[hnb;q]
            pc = pcs.tile([128, 512], dt.float32)   # c state L0|L1 (rows 0-63)

            nc.sync.dma_start(WT[:], wts)
            nc.vector.memset(Hh[0:64, :], 0.0)
            nc.vector.memset(Hh[64:66, :], 1.0)     # ones + x slot
            nc.vector.memset(HG[:, :], 0.0)         # zero-blocks load-bearing
            nc.vector.memset(pc[0:64, :], 0.0)      # c state init
            nc.vector.memset(Ut[0:64, :], 0.0)      # c fp16 mirror init

            for s in range(STEPS):
                sigL = psigL.tile([128, 1024], dt.float32, tag="sigL")
                sigG = psigG.tile([128, 512], dt.float32, tag="sigG")
                SHL = work.tile([128, 1024], dt.float16, tag="shl")
                SHG = work.tile([128, 512], dt.float16, tag="shg")
                PtL = work.tile([128, 512], dt.float16, tag="ptl")
                THc = work.tile([64, 512], dt.float16, tag="thc")
                THn = work.tile([128, 512], dt.float16, tag="thn")
                Wz = work.tile([128, 512], dt.float16, tag="wz")
                Zh = work.tile([128, 512], dt.float16, tag="zh")
                Pn = work.tile([128, 512], dt.float16, tag="pn")

                # x for step s rides Hh row 65 ([ones; x] DMA'd to rows 64-65)
                nc.sync.dma_start(Hh[64:66, 0:256], xt[2 * s : 2 * s + 2, :])

                mm = nc.tensor.matmul
                # --- merged GRU self matmuls (block-diag over [g0h; g1h]) ---
                mm(sigG[:, 0:512], WT[blk(8, 128)], HG[:, :],
                   start=True, stop=False, skip_group_check=True)
                mm(gq[:, 0:512], WT[blk(10, 128)], HG[:, :],
                   start=True, stop=False, skip_group_check=True)

                # --- LSTM gate matmuls: the h_L recurrence is critical ---
                mm(sigL[:, 0:256], WT[blk(0, 66)], Hh[0:66, 0:256],
                   start=True, stop=True, skip_group_check=True)
                mm(sigL[:, 256:512], WT[blk(4, 64)], Hh[0:64, 256:512],
                   start=False, stop=False, skip_group_check=True)
                mm(sigL[:, 256:512], WT[blk(5, 65)], Hh[0:65, 0:256],
                   start=False, stop=True, skip_group_check=True)
                mm(sigL[:, 512:768], WT[blk(2, 66)], Hh[0:66, 0:256],
                   start=True, stop=True, skip_group_check=True)
                mm(sigL[:, 768:1024], WT[blk(6, 64)], Hh[0:64, 256:512],
                   start=False, stop=False, skip_group_check=True)
                mm(sigL[:, 768:1024], WT[blk(7, 65)], Hh[0:65, 0:256],
                   start=False, stop=True, skip_group_check=True)

                # --- GRU chunk-A below matmuls ---
                mm(sigG[:, 0:256], WT[blk(9, 65)], Hh[0:65, 256:512],
                   start=False, stop=True, skip_group_check=True)
                mm(sigG[:, 256:512], WT[blk(13, 65)], Hh[0:65, 512:768],
                   start=False, stop=True, skip_group_check=True)

                # --- activations in readiness order ---
                nc.scalar.activation(SHL[:, 0:512], sigL[:, 0:512], Act.Sigmoid)
                nc.scalar.activation(Ut[64:128, :], sigL[64:128, 512:1024],
                                     Act.Tanh)
                nc.scalar.activation(SHL[0:64, 512:1024], sigL[0:64, 512:1024],
                                     Act.Sigmoid)
                nc.scalar.activation(SHG[:], sigG[:, :], Act.Sigmoid)

                # --- LSTM element-wise: [f*c; i*u] in one 2x fp16 op ---
                nc.vector.tensor_tensor(PtL[:, :], SHL[:, 0:512],
                                        Ut[:, :], Alu.mult)

                # c' = f*c + i*u (cross-partition fold on PE)
                mm(pc[0:64, :], WT[blk(16, 128, 64)], PtL[:, :],
                   start=True, stop=True, skip_group_check=True)
                nc.scalar.activation(THc[:, :], pc[0:64, :], Act.Tanh)
                # evacuate c' to the fp16 [c; u] tile for the next step
                nc.vector.tensor_copy(Ut[0:64, :], pc[0:64, :])

                # --- GRU chunk-B below matmuls (feed PtG/NSEL later) ---
                mm(gq[:, 0:256], WT[blk(11, 65)], Hh[0:65, 256:512],
                   start=False, stop=False, skip_group_check=True)
                mm(gq[:, 256:512], WT[blk(15, 65)], Hh[0:65, 512:768],
                   start=False, stop=False, skip_group_check=True)

                # --- GRU element-wise: t = m*hnb; n_pre = q - t ---
                # meaningful: G0 rows 64-127, G1 rows 0-63 (flipped layout)
                nc.vector.tensor_tensor(PtG[:, :], SHG[:, :],
                                        gq[:, 0:512], Alu.mult)
                mm(gq[:, 0:512], WT[blk(17, 128, 128)], PtG[:, :],
                   start=False, stop=True, skip_group_check=True)
                nc.scalar.activation(THn[:, :], gq[:, 0:512], Act.Tanh)

                # h_lstm = o * tanh(c)  (feeds the critical loop first)
                nc.vector.tensor_tensor(Hh[0:64, 0:512], SHL[0:64, 512:1024],
                                        THc[:, :], Alu.mult)
                # GRU tail as h' = n*(1-z) + z*h: (1-z) and z*h only need
                # sigma(z) + old h, so they run early, OFF the tanh_n path;
                # only two ops remain after tanh_n. Off-quadrant lanes
                # compute finite garbage; never read.
                nc.vector.tensor_scalar(Wz[:, :], SHG[:, :],
                                        -1.0, 1.0, Alu.mult, Alu.add)
                nc.vector.tensor_tensor(Zh[:, :], SHG[:, :],
                                        HG[:, :], Alu.mult)
                nc.vector.tensor_tensor(Pn[:, :], THn[:, :],
                                        Wz[:, :], Alu.mult)
                nc.vector.tensor_tensor(HG[0:64, 0:256], Pn[0:64, 0:256],
                                        Zh[0:64, 0:256], Alu.add)
                nc.vector.tensor_tensor(HG[64:128, 256:512],
                                        Pn[64:128, 256:512],
                                        Zh[64:128, 256:512], Alu.add)
                # G1 reads g0h (+ones) from Hh; keep that copy in sync
                nc.vector.tensor_copy(Hh[0:64, 512:768], HG[0:64, 0:256])

                # wavefront warm-up: re-zero states of layers not yet active
                if s == 0:
                    nc.vector.memset(Hh[0:64, 256:512], 0.0)      # h1
                    nc.vector.memset(pc[0:64, 256:512], 0.0)      # c1
                    nc.vector.memset(Ut[0:64, 256:512], 0.0)      # c1 mirror
                elif s == 1:
                    nc.vector.memset(Hh[0:64, 512:768], 0.0)      # g0h
                    nc.vector.memset(HG[0:64, 0:256], 0.0)
                elif s == 2:
                    nc.vector.memset(HG[64:128, 256:512], 0.0)    # g1h

            # --- FC head: y = g1h(T-1) @ fc_w.T + fc_b ---
            ysb = state.tile([1, 256], dt.float32)
            nc.tensor.matmul(pc[0:1, 0:256], WT[blk(18, 128, 1)],
                             HG[:, 256:512],
                             start=True, stop=False, skip_group_check=True)
            nc.tensor.matmul(pc[0:1, 0:256],
                             WT[64:65, 128 * 12 : 128 * 12 + 1],
                             Hh[64:65, 0:256],
                             start=False, stop=True, skip_group_check=True)
            nc.vector.tensor_copy(ysb[:], pc[0:1, 0:256])
            nc.sync.dma_start(y, ysb[:])

    nc.compile()
    return nc


_NC_CACHE = {}


def _get_nc(T):
    if T not in _NC_CACHE:
        _NC_CACHE[T] = build_kernel(T)
    return _NC_CACHE[T]


def _prep_inputs(inputs, T):
    from concourse import bass_utils  # noqa: F401

    x = np.asarray(inputs["x"], np.float32)  # [2048, T_FULL, 1]
    W = _pack_weights(inputs)
    STEPS = T + 3
    in_maps = []
    for c in range(N_CORES):
        xc = x[c * B_CORE : (c + 1) * B_CORE, :T, 0]      # [256, T]
        xtc = np.zeros((STEPS, 2, 256), f16)
        xtc[:, 0, :] = 1.0                                # ones row refresh
        xtc[:T, 1, :] = xc.T.astype(f16)
        in_maps.append({"xt": xtc.reshape(2 * STEPS, 256), "wts": W})
    return in_maps


def run(inputs, T):
    from concourse import bass_utils

    nc = _get_nc(T)
    in_maps = _prep_inputs(inputs, T)
    res = bass_utils.run_bass_kernel_spmd(nc, in_maps, core_ids=list(range(N_CORES)))
    out = np.concatenate([res.results[c]["y"].reshape(256, 1) for c in range(N_CORES)], axis=0)
    return out.astype(np.float32)


def kernel(**inputs):
    return run(inputs, T_FULL)



# revision 57
# speedup vs baseline: 1.2845x; 1.0535x over previous
"""Trainium2 Bass kernel for a 4-layer RNN stack (LSTM x2 + GRU x2, H=64) + FC head.

Strategy: data-parallel over batch (B=2048 -> 256/core on 8 cores). On each
core, all four layers run as a software wavefront over time: at global step s,
layer L processes timestep t = s - L. All state lives on-chip.

Per step: 14 gate matmuls (self/below K-split; L0's x and all biases ride
extra rhs rows — x is DMA'd into Hh rows 64-65 as [ones; x] each step),
per-chunk sigmoids plus a real tanh for the LSTM g-gate, one combined
[f*c; i*u] fp16 DVE multiply against a persistent [c; u] SBUF tile, and
identity/selector matmuls for the cross-partition folds (LSTM c' = f*c + i*u,
GRU n_pre = q - m*hnb). Engine queues are ordered so the h_L recurrence (the
critical loop) threads first; GRU self-MMs fill the PE while the LSTM tail
runs. Gate pre-activations accumulate in fp32 PSUM; element-wise state fp16.
"""

import numpy as np

H = 64
B_CORE = 256
T_FULL = 512
N_CORES = 8
NBLK = 19  # weight blocks of 128 cols each

f16 = np.float16


# ---------------------------------------------------------------------------
# Host-side weight packing
# ---------------------------------------------------------------------------
def _pack_weights(inp):
    """Build the [128, NBLK*128] fp16 lhsT table. Block j lives at cols 128j.

    LSTM chunk A = [f; i] rows, chunk B = [o; 2*g] rows (sigma(2g) trick).
    GRU  chunk A = [z; -r] rows, chunk B = [q; hnb] (q = xn-part + hn-part).
    """
    W = np.zeros((128, NBLK * 128), np.float32)

    def put(j, arr):
        a = np.asarray(arr, np.float32)
        W[: a.shape[0], 128 * j : 128 * j + a.shape[1]] = a

    def lstm_blocks(w_ih, w_hh, b_ih, b_hh, j0, first):
        b = np.asarray(b_ih, np.float32) + np.asarray(b_hh, np.float32)
        w_ih = np.asarray(w_ih, np.float32)
        w_hh = np.asarray(w_hh, np.float32)
        rA = list(range(64, 128)) + list(range(0, 64))      # [f; i]
        rB = list(range(192, 256)) + list(range(128, 192))  # [o; g]
        for k, rows in enumerate([rA, rB]):
            whh = w_hh[rows]
            wih = w_ih[rows]
            bb = b[rows]
            if first:
                # L0: one merged MM; x rides Hh row 65 (ones stays at 64)
                xr = wih[:, 0][None, :]
                put(j0 + 2 * k, np.vstack([whh.T, bb[None, :], xr]))  # [66,128]
            else:
                put(j0 + 2 * k, whh.T)                                # [64,128]
                put(j0 + 2 * k + 1, np.vstack([wih.T, bb[None, :]]))  # [65,128]

    def gru_parts(w_ih, w_hh, b_ih, b_hh, flip):
        """Return (selfA, belowA+bias, selfB, belowB+bias) for one GRU.

        Normal (G0): A cols = [z; -r], B cols = [q; hnb], row-half LOW.
        Flipped (G1): A cols = [-r; z], B cols = [hnb; q], row-half HIGH.
        """
        w_ih = np.asarray(w_ih, np.float32)
        w_hh = np.asarray(w_hh, np.float32)
        b_ih = np.asarray(b_ih, np.float32)
        b_hh = np.asarray(b_hh, np.float32)
        r, z, n = slice(0, 64), slice(64, 128), slice(128, 192)
        zc = (w_hh[z], w_ih[z], b_ih[z] + b_hh[z])
        rc = (-w_hh[r], -w_ih[r], -(b_ih[r] + b_hh[r]))
        qc = (w_hh[n], w_ih[n], b_ih[n] + b_hh[n])
        hc = (w_hh[n], np.zeros_like(w_ih[n]), b_hh[n])
        a0, a1 = (rc, zc) if flip else (zc, rc)
        b0, b1 = (hc, qc) if flip else (qc, hc)
        selfA = np.concatenate([a0[0], a1[0]], axis=0).T
        belowA = np.vstack([np.concatenate([a0[1], a1[1]], axis=0).T,
                            np.concatenate([a0[2], a1[2]])[None, :]])
        selfB = np.concatenate([b0[0], b1[0]], axis=0).T
        belowB = np.vstack([np.concatenate([b0[1], b1[1]], axis=0).T,
                            np.concatenate([b0[2], b1[2]])[None, :]])
        return selfA, belowA, selfB, belowB

    lstm_blocks(inp["lw_ih0"], inp["lw_hh0"], inp["lb_ih0"], inp["lb_hh0"], 0, True)
    lstm_blocks(inp["lw_ih1"], inp["lw_hh1"], inp["lb_ih1"], inp["lb_hh1"], 4, False)
    sA0, bA0, sB0, bB0 = gru_parts(
        inp["gw_ih0"], inp["gw_hh0"], inp["gb_ih0"], inp["gb_hh0"], False)
    sA1, bA1, sB1, bB1 = gru_parts(
        inp["gw_ih1"], inp["gw_hh1"], inp["gb_ih1"], inp["gb_hh1"], True)
    # merged self blocks: rows 0-63 contract g0h (LOW), 64-127 g1h (HIGH)
    put(8, np.vstack([sA0, sA1]))             # [128,128]
    put(9, bA0)                               # [65,128] G0A below+bias
    put(10, np.vstack([sB0, sB1]))            # [128,128]
    put(11, bB0)                              # [65,128] G0B below+bias
    put(13, bA1)                              # [65,128] G1A below+bias
    put(15, bB1)                              # [65,128] G1B below+bias

    eye = np.eye(64, dtype=np.float32)
    put(16, np.vstack([eye, eye]))            # IDT2: out[m] = x[m] + x[m+64]
    z64 = np.zeros((64, 64), np.float32)
    # NSEL2: out[m<64] = -x[m+64], out[m>=64] = -x[m-64]
    put(17, np.block([[z64, -eye], [-eye, z64]]))
    fc = np.zeros((128, 128), np.float32)
    fc[64:128, 0] = np.asarray(inp["fc_w"], np.float32)[0]  # g1h on HIGH rows
    put(18, fc)
    fcb = np.zeros((65, 128), np.float32)
    fcb[64, 0] = np.asarray(inp["fc_b"], np.float32)[0]     # rides ones row
    put(12, fcb)
    return W.astype(f16)


# ---------------------------------------------------------------------------
# Bass kernel
# ---------------------------------------------------------------------------
def build_kernel(T):
    import concourse.bass as bass
    import concourse.tile as tile
    from concourse import mybir, bacc

    dt = mybir.dt
    Alu = mybir.AluOpType
    Act = mybir.ActivationFunctionType
    STEPS = T + 3

    nc = bacc.Bacc(None, target_bir_lowering=False, debug=False)
    xt = nc.dram_tensor("xt", [2 * STEPS, 256], dt.float16, kind="ExternalInput").ap()
    wts = nc.dram_tensor("wts", [128, NBLK * 128], dt.float16, kind="ExternalInput").ap()
    y = nc.dram_tensor("y", [1, 256], dt.float32, kind="ExternalOutput").ap()

    def blk(j, k=128, w=128):
        return (slice(0, k), slice(128 * j, 128 * j + w))

    with tile.TileContext(nc) as tc:
        with (
            tc.tile_pool(name="state", bufs=1) as state,
            tc.tile_pool(name="work", bufs=2) as work,
            tc.tile_pool(name="psigL", bufs=1, space="PSUM") as psigL,
            tc.tile_pool(name="psigG", bufs=2, space="PSUM") as psigG,
            tc.tile_pool(name="pgq", bufs=2, space="PSUM") as pgq,
            tc.tile_pool(name="pcs", bufs=1, space="PSUM") as pcs,
        ):
            WT = state.tile([128, NBLK * 128], dt.float16)
            Hh = state.tile([66, 1024], dt.float16)
            HG = state.tile([128, 512], dt.float16)  # [g0h|0 ; 0|g1h]
            PtG = state.tile([128, 512], dt.float16)
            Ut = state.tile([128, 512], dt.float16)  # rows 0-63: c; 64-127: u
            gq = pgq.tile([128, 512], dt.float32)   # G0:[q;hnb] | G1:[hnb;q]
            pc = pcs.tile([128, 512], dt.float32)   # c state L0|L1 (rows 0-63)

            nc.sync.dma_start(WT[:], wts)
            nc.vector.memset(Hh[0:64, :], 0.0)
            nc.vector.memset(Hh[64:66, :], 1.0)     # ones + x slot
            nc.vector.memset(HG[:, :], 0.0)         # zero-blocks load-bearing
            nc.vector.memset(pc[0:64, :], 0.0)      # c state init
            nc.vector.memset(Ut[0:64, :], 0.0)      # c fp16 mirror init

            for s in range(STEPS):
                sigL = psigL.tile([128, 1024], dt.float32, tag="sigL")
                sigG = psigG.tile([128, 512], dt.float32, tag="sigG")
                SHL = work.tile([128, 1024], dt.float16, tag="shl")
                SHG = work.tile([128, 512], dt.float16, tag="shg")
                PtL = work.tile([128, 512], dt.float16, tag="ptl")
                THc = work.tile([64, 512], dt.float16, tag="thc")
                THn = work.tile([128, 512], dt.float16, tag="thn")
                Wz = work.tile([128, 512], dt.float16, tag="wz")
                Zh = work.tile([128, 512], dt.float16, tag="zh")
                Pn = work.tile([128, 512], dt.float16, tag="pn")

                # x for step s rides Hh row 65 ([ones; x] DMA'd to rows 64-65)
                nc.sync.dma_start(Hh[64:66, 0:256], xt[2 * s : 2 * s + 2, :])

                mm = nc.tensor.matmul
                # --- merged GRU self matmuls (block-diag over [g0h; g1h]) ---
                mm(sigG[:, 0:512], WT[blk(8, 128)], HG[:, :],
                   start=True, stop=False, skip_group_check=True)
                mm(gq[:, 0:512], WT[blk(10, 128)], HG[:, :],
                   start=True, stop=False, skip_group_check=True)

                # --- LSTM gate matmuls: the h_L recurrence is critical ---
                mm(sigL[:, 0:256], WT[blk(0, 66)], Hh[0:66, 0:256],
                   start=True, stop=True, skip_group_check=True)
                mm(sigL[:, 256:512], WT[blk(4, 64)], Hh[0:64, 256:512],
                   start=False, stop=False, skip_group_check=True)
                mm(sigL[:, 256:512], WT[blk(5, 65)], Hh[0:65, 0:256],
                   start=False, stop=True, skip_group_check=True)
                mm(sigL[:, 512:768], WT[blk(2, 66)], Hh[0:66, 0:256],
                   start=True, stop=True, skip_group_check=True)
                mm(sigL[:, 768:1024], WT[blk(6, 64)], Hh[0:64, 256:512],
                   start=False, stop=False, skip_group_check=True)
                mm(sigL[:, 768:1024], WT[blk(7, 65)], Hh[0:65, 0:256],
                   start=False, stop=True, skip_group_check=True)

                # --- GRU chunk-A below matmuls ---
                mm(sigG[:, 0:256], WT[blk(9, 65)], Hh[0:65, 256:512],
                   start=False, stop=True, skip_group_check=True)
                mm(sigG[:, 256:512], WT[blk(13, 65)], Hh[0:65, 512:768],
                   start=False, stop=True, skip_group_check=True)

                # --- activations in readiness order ---
                nc.scalar.activation(SHL[:, 0:512], sigL[:, 0:512], Act.Sigmoid)
                nc.scalar.activation(Ut[64:128, :], sigL[64:128, 512:1024],
                                     Act.Tanh)
                nc.scalar.activation(SHL[0:64, 512:1024], sigL[0:64, 512:1024],
                                     Act.Sigmoid)
                nc.scalar.activation(SHG[:], sigG[:, :], Act.Sigmoid)

                # --- LSTM element-wise: [f*c; i*u] in one 2x fp16 op ---
                nc.vector.tensor_tensor(PtL[:, :], SHL[:, 0:512],
                                        Ut[:, :], Alu.mult)

                # c' = f*c + i*u (cross-partition fold on PE), per layer so
                # each layer's tanh/h chain fires as soon as its half lands
                mm(pc[0:64, 0:256], WT[blk(16, 128, 64)], PtL[:, 0:256],
                   start=True, stop=True, skip_group_check=True)
                nc.scalar.activation(THc[:, 0:256], pc[0:64, 0:256], Act.Tanh)
                mm(pc[0:64, 256:512], WT[blk(16, 128, 64)], PtL[:, 256:512],
                   start=False, stop=True, skip_group_check=True)
                nc.scalar.activation(THc[:, 256:512], pc[0:64, 256:512],
                                     Act.Tanh)

                # --- GRU chunk-B below matmuls (feed PtG/NSEL later) ---
                mm(gq[:, 0:256], WT[blk(11, 65)], Hh[0:65, 256:512],
                   start=False, stop=False, skip_group_check=True)
                mm(gq[:, 256:512], WT[blk(15, 65)], Hh[0:65, 512:768],
                   start=False, stop=False, skip_group_check=True)

                # --- GRU element-wise: t = m*hnb; n_pre = q - t ---
                # meaningful: G0 rows 64-127, G1 rows 0-63 (flipped layout)
                nc.vector.tensor_tensor(PtG[:, :], SHG[:, :],
                                        gq[:, 0:512], Alu.mult)
                mm(gq[:, 0:512], WT[blk(17, 128, 128)], PtG[:, :],
                   start=False, stop=True, skip_group_check=True)
                nc.scalar.activation(THn[:, :], gq[:, 0:512], Act.Tanh)

                # h_lstm = o * tanh(c), per layer (h0 lands without waiting
                # for L1's fold)
                nc.vector.tensor_tensor(Hh[0:64, 0:256], SHL[0:64, 512:768],
                                        THc[:, 0:256], Alu.mult)
                nc.vector.tensor_tensor(Hh[0:64, 256:512], SHL[0:64, 768:1024],
                                        THc[:, 256:512], Alu.mult)
                # evacuate c' to the fp16 [c; u] tile (consumer is next step's
                # PtL, so keep it behind h_lstm in the DVE queue)
                nc.vector.tensor_copy(Ut[0:64, :], pc[0:64, :])
                # GRU tail as h' = n*(1-z) + z*h: (1-z) and z*h only need
                # sigma(z) + old h, so they run early, OFF the tanh_n path;
                # only two ops remain after tanh_n. Off-quadrant lanes
                # compute finite garbage; never read.
                nc.vector.tensor_scalar(Wz[:, :], SHG[:, :],
                                        -1.0, 1.0, Alu.mult, Alu.add)
                nc.vector.tensor_tensor(Zh[:, :], SHG[:, :],
                                        HG[:, :], Alu.mult)
                nc.vector.tensor_tensor(Pn[:, :], THn[:, :],
                                        Wz[:, :], Alu.mult)
                nc.vector.tensor_tensor(HG[0:64, 0:256], Pn[0:64, 0:256],
                                        Zh[0:64, 0:256], Alu.add)
                nc.vector.tensor_tensor(HG[64:128, 256:512],
                                        Pn[64:128, 256:512],
                                        Zh[64:128, 256:512], Alu.add)
                # G1 reads g0h (+ones) from Hh; keep that copy in sync
                nc.vector.tensor_copy(Hh[0:64, 512:768], HG[0:64, 0:256])

                # wavefront warm-up: re-zero states of layers not yet active
                if s == 0:
                    nc.vector.memset(Hh[0:64, 256:512], 0.0)      # h1
                    nc.vector.memset(pc[0:64, 256:512], 0.0)      # c1
                    nc.vector.memset(Ut[0:64, 256:512], 0.0)      # c1 mirror
                elif s == 1:
                    nc.vector.memset(Hh[0:64, 512:768], 0.0)      # g0h
                    nc.vector.memset(HG[0:64, 0:256], 0.0)
                elif s == 2:
                    nc.vector.memset(HG[64:128, 256:512], 0.0)    # g1h

            # --- FC head: y = g1h(T-1) @ fc_w.T + fc_b ---
            ysb = state.tile([1, 256], dt.float32)
            nc.tensor.matmul(pc[0:1, 0:256], WT[blk(18, 128, 1)],
                             HG[:, 256:512],
                             start=True, stop=False, skip_group_check=True)
            nc.tensor.matmul(pc[0:1, 0:256],
                             WT[64:65, 128 * 12 : 128 * 12 + 1],
                             Hh[64:65, 0:256],
                             start=False, stop=True, skip_group_check=True)
            nc.vector.tensor_copy(ysb[:], pc[0:1, 0:256])
            nc.sync.dma_start(y, ysb[:])

    nc.compile()
    return nc


_NC_CACHE = {}


def _get_nc(T):
    if T not in _NC_CACHE:
        _NC_CACHE[T] = build_kernel(T)
    return _NC_CACHE[T]


def _prep_inputs(inputs, T):
    from concourse import bass_utils  # noqa: F401

    x = np.asarray(inputs["x"], np.float32)  # [2048, T_FULL, 1]
    W = _pack_weights(inputs)
    STEPS = T + 3
    in_maps = []
    for c in range(N_CORES):
        xc = x[c * B_CORE : (c + 1) * B_CORE, :T, 0]      # [256, T]
        xtc = np.zeros((STEPS, 2, 256), f16)
        xtc[:, 0, :] = 1.0                                # ones row refresh
        xtc[:T, 1, :] = xc.T.astype(f16)
        in_maps.append({"xt": xtc.reshape(2 * STEPS, 256), "wts": W})
    return in_maps


def run(inputs, T):
    from concourse import bass_utils

    nc = _get_nc(T)
    in_maps = _prep_inputs(inputs, T)
    res = bass_utils.run_bass_kernel_spmd(nc, in_maps, core_ids=list(range(N_CORES)))
    out = np.concatenate([res.results[c]["y"].reshape(256, 1) for c in range(N_CORES)], axis=0)
    return out.astype(np.float32)


def kernel(**inputs):
    return run(inputs, T_FULL)

